# revision 15
# baseline (speedup 1.0000x reference)
"""Trainium2 Bass kernel for EnhancedGATModel (3-layer GATv2, N=50000, E=800000).

v2 strategy (8 NeuronCores, graph-partitioned by destination node):
- Host: append self-loops, sort edges by dst, partition dst nodes 6250/core,
  bucket edges per 128-dst block, split each block's edges by src half
  (int16 gather indices), pad to 128-edge tiles uniformly across cores.
- Device (single SPMD NEFF on 8 cores), all-bf16 edge path:
  * per-layer node tables xl = h@Wl (own shard, bf16) + AllGather -> full
    table; xr tables and h residuals stay SBUF-resident (never leave chip)
  * per tile: dma_gather of xl[src] rows (bf16); one-hot (iota==dstl) on DVE;
    xr[dst] rows reconstructed on the PE via ohT@xr_block (no xr gather);
    u = xr_e + xl_s accumulated in PSUM; Prelu on Act; per-head logits via
    mult+reduce on DVE; exp on Act; weighted scatter-add via one-hot
    matmul accumulating [num|den] in PSUM across the block's tiles
  * block post: normalize, BN affine (folded), relu, residual; next layer's
    node prep (xl/xr matmuls) interleaved per block; final log_softmax.
"""
import sys
import numpy as np
import ml_dtypes

sys.path.insert(0, "/opt/trn_rl_repo")

import concourse.bass as bass
import concourse.mybir as mybir
import concourse.tile as tile
from concourse import bacc
from concourse.bass_utils import run_bass_kernel_spmd

F32 = mybir.dt.float32
BF16 = mybir.dt.bfloat16
I16 = mybir.dt.int16
AF = mybir.ActivationFunctionType
ALU = mybir.AluOpType
BFNP = ml_dtypes.bfloat16

NCORES = 8
BLOCK = 128
D_IN, HID, HEADS, OUT = 128, 64, 4, 2
HC = HEADS * HID  # 256
NEG_SLOPE = 0.2
BN_EPS = 1e-5
GMAX = 8  # dma_gather limit: <=1024 indices per op
PRELU_VIA_DVE = False  # CoreSim lacks Prelu; flip for simulation


def _prelu(nc, sb, out_ap, in_ap, shape):
    """out = prelu(in). Act Prelu on hw; mult+max on DVE for CoreSim."""
    if not PRELU_VIA_DVE:
        nc.scalar.activation(out_ap, in_ap, AF.Prelu, alpha=NEG_SLOPE)
    else:
        w = sb.tile(shape, F32, tag="pw")
        nc.vector.tensor_scalar(out=w[:], in0=in_ap, scalar1=NEG_SLOPE,
                                scalar2=None, op0=ALU.mult)
        nc.vector.tensor_tensor(out=out_ap, in0=w[:], in1=in_ap, op=ALU.max)


# ---------------------------------------------------------------- host prep
def preprocess(edge_index, N):
    """Per-core gather index / dst-local arrays and the tile schedule.

    schedule: list of (block, 'lo'|'hi', ntiles) in tile order; uniform
    across cores. Edge k of a (core,block,half) group lands at partition
    k%128 of tile k//128; pads use src-index 0 (finite reads) and
    dst_local=200 (masked out of the one-hot).
    """
    NPC = N // NCORES
    NBLK = (NPC + BLOCK - 1) // BLOCK
    HALF = N // 2
    src = np.concatenate([edge_index[0], np.arange(N)]).astype(np.int64)
    dst = np.concatenate([edge_index[1], np.arange(N)]).astype(np.int64)
    order = np.argsort(dst, kind="stable")
    src, dst = src[order], dst[order]
    core_of = dst // NPC
    groups = {}
    for c in range(NCORES):
        m = core_of == c
        sc, dc = src[m], dst[m]
        loc = dc - c * NPC
        blk = loc // BLOCK
        lo = sc < HALF
        for b in range(NBLK):
            mb = blk == b
            groups[(c, b, 0)] = (sc[mb & lo], loc[mb & lo] % BLOCK)
            groups[(c, b, 1)] = (sc[mb & ~lo] - HALF, loc[mb & ~lo] % BLOCK)
    schedule = []
    for b in range(NBLK):
        for h, nm in ((0, "lo"), (1, "hi")):
            mx = max(len(groups[(c, b, h)][0]) for c in range(NCORES))
            T = (mx + 127) // 128
            if T > 0:
                schedule.append((b, nm, T))
    TT = sum(T for _, _, T in schedule)
    idx_xl = np.zeros((NCORES, 128, 8 * TT), np.int16)
    dstl = np.full((NCORES, 128, TT), 200.0, np.float32)
    t0 = 0
    for b, nm, T in schedule:
        h = 0 if nm == "lo" else 1
        for c in range(NCORES):
            s, dl = groups[(c, b, h)]
            ne = len(s)
            pad = T * 128 - ne
            sp = np.concatenate([s, np.zeros(pad, np.int64)]).astype(np.int64)
            dlp = np.concatenate([dl, np.full(pad, 200)]).astype(np.int64)
            wrap = sp.reshape(8 * T, 16).T.astype(np.int16)
            idx_xl[c, :, 8 * t0:8 * (t0 + T)] = np.tile(wrap, (8, 1))
            dstl[c, :, t0:t0 + T] = dlp.reshape(T, 128).T.astype(np.float32)
        t0 += T
    return idx_xl, dstl, schedule, NBLK


def pack_consts(ip):
    """Pack constants into cf (f32) and cb (bf16) [128, *] tensors.
    Returns (cf_arr, cb_arr, dict name -> ('f'|'b', rows, col0, cols))."""
    cols = {}
    fparts, bparts = [], []
    fc, bc = [0], [0]

    def addf(name, arr):
        a = np.zeros((128, arr.shape[1]), np.float32)
        a[:arr.shape[0]] = arr
        cols[name] = ("f", arr.shape[0], fc[0], arr.shape[1])
        fparts.append(a)
        fc[0] += arr.shape[1]

    def addb(name, arr):
        a = np.zeros((128, arr.shape[1]), np.float32)
        a[:arr.shape[0]] = arr
        cols[name] = ("b", arr.shape[0], bc[0], arr.shape[1])
        bparts.append(a)
        bc[0] += arr.shape[1]

    bcast = lambda v: np.broadcast_to(
        np.asarray(v, np.float32)[None, :], (128, len(np.asarray(v)))).copy()

    # --- bf16 consts
    addb("ident", np.eye(128, dtype=np.float32))
    iota = np.broadcast_to(np.arange(128, dtype=np.float32), (128, 128))
    addb("iotaB", np.ascontiguousarray(iota))
    addb("attB0", bcast(np.asarray(ip["att0"], np.float32).reshape(-1)))
    addb("attB1", bcast(np.asarray(ip["att1"], np.float32).reshape(-1)))
    addb("attB2", bcast(np.asarray(ip["att2"], np.float32).reshape(-1)))
    g, bt = np.asarray(ip["bn_gamma"]), np.asarray(ip["bn_beta"])
    mu, var = np.asarray(ip["bn_mean"]), np.asarray(ip["bn_var"])
    for l in range(2):
        a = g[l] / np.sqrt(var[l] + BN_EPS)
        bias_l = np.asarray(ip[f"bias{l}"], np.float32)
        b = bt[l] - mu[l] * a + a * bias_l
        addb(f"aB{l}", bcast(a))
        addb(f"bB{l}", bcast(b))
    addb("W_in", np.asarray(ip["W_in"], np.float32))          # [128, 64]
    addb("Wl0", np.asarray(ip["Wl0"], np.float32))            # [64, 256]
    addb("Wr0", np.asarray(ip["Wr0"], np.float32))
    Wl1 = np.asarray(ip["Wl1"], np.float32)
    Wr1 = np.asarray(ip["Wr1"], np.float32)
    addb("Wl1k0", Wl1[:128]); addb("Wl1k1", Wl1[128:])
    addb("Wr1k0", Wr1[:128]); addb("Wr1k1", Wr1[128:])
    Wl2 = np.asarray(ip["Wl2"], np.float32)
    Wr2 = np.asarray(ip["Wr2"], np.float32)
    addb("Wl2k0", Wl2[:128]); addb("Wl2k1", Wl2[128:])
    addb("Wr2k0", Wr2[:128]); addb("Wr2k1", Wr2[128:])
    # --- f32 consts
    addf("b_in", np.asarray(ip["b_in"], np.float32).reshape(-1, 1))  # [64,1]
    addf("bias2F", bcast(np.asarray(ip["bias2"], np.float32)))       # [128,2]

    cf = np.concatenate(fparts, axis=1) if fparts else np.zeros((128, 1), np.float32)
    cb = np.concatenate(bparts, axis=1)
    return cf, cb.astype(BFNP), cols


def _gather(nc, out_tile, in_ap, idx_tile, tstart, T, elem):
    """Chunked dma_gather: out_tile[:, k:k+Tc, :] = table[idx[tile k..]]."""
    k = 0
    while k < T:
        Tc = min(GMAX, T - k)
        nc.gpsimd.dma_gather(
            out_ap=out_tile[:, k:k + Tc, :], in_ap=in_ap,
            idxs_ap=idx_tile[:, 8 * (tstart + k):8 * (tstart + k + Tc)],
            num_idxs=128 * Tc, num_idxs_reg=128 * Tc, elem_size=elem)
        k += Tc


def build(N, schedule, NBLK, TT, CWF, CWB):
    NPC = N // NCORES
    HALF = N // 2
    nc = bacc.Bacc("TRN2", target_bir_lowering=False, debug=False)

    xT = nc.dram_tensor("xT", [D_IN, NPC], BF16, kind="ExternalInput")
    idx_xl = nc.dram_tensor("idx_xl", [128, 8 * TT], I16, kind="ExternalInput")
    dstl = nc.dram_tensor("dstl", [128, TT], F32, kind="ExternalInput")
    cfd = nc.dram_tensor("cf", [128, CWF], F32, kind="ExternalInput")
    cbd = nc.dram_tensor("cb", [128, CWB], BF16, kind="ExternalInput")
    out = nc.dram_tensor("out", [NPC, OUT], F32, kind="ExternalOutput")

    xl0_own = nc.dram_tensor("xl0_own", [NPC, HC], BF16)
    xl0_full = nc.dram_tensor("xl0_full", [N, HC], BF16, addr_space="Shared")
    xl1_own = nc.dram_tensor("xl1_own", [NPC, HC], BF16)
    xl1_full = nc.dram_tensor("xl1_full", [N, HC], BF16, addr_space="Shared")
    xl2_own = nc.dram_tensor("xl2_own", [NPC, 128], BF16)
    xl2_full = nc.dram_tensor("xl2_full", [N, 128], BF16, addr_space="Shared")

    # block tile layout: per block b, list of (tile_start, 'lo'|'hi', T)
    blk_tiles = {b: [] for b in range(NBLK)}
    t0 = 0
    for b, nm, T in schedule:
        blk_tiles[b].append((t0, nm, T))
        t0 += T

    rg = [list(range(NCORES))]

    with tile.TileContext(nc) as tc:
        import contextlib
        with contextlib.ExitStack() as ctx:
            cst = ctx.enter_context(tc.tile_pool(name="cst", bufs=1))
            sb = ctx.enter_context(tc.tile_pool(name="sb", bufs=3))
            gat = ctx.enter_context(tc.tile_pool(name="gat", bufs=2))
            ps = ctx.enter_context(tc.tile_pool(name="ps", bufs=2, space="PSUM"))
            psu = ctx.enter_context(tc.tile_pool(name="psu", bufs=2, space="PSUM"))
            pst = ctx.enter_context(tc.tile_pool(name="pst", bufs=2, space="PSUM"))
            psa = ctx.enter_context(tc.tile_pool(name="psa", bufs=2, space="PSUM"))

            CF = cst.tile([128, CWF], F32)
            nc.sync.dma_start(CF[:], cfd[:])
            CB = cst.tile([128, CWB], BF16)
            nc.sync.dma_start(CB[:], cbd[:])
            ixl_t = cst.tile([128, 8 * TT], I16)
            nc.sync.dma_start(ixl_t[:], idx_xl[:])
            dstl_t = cst.tile([128, TT], F32)
            nc.sync.dma_start(dstl_t[:], dstl[:])

            xr0_sb = cst.tile([128, NBLK, HC], BF16)
            xr1_sb = cst.tile([128, NBLK, HC], BF16)
            xr2_sb = cst.tile([128, NBLK, OUT], BF16)
            h1_sb = cst.tile([128, NBLK, HC], BF16)
            h2_sb = cst.tile([128, NBLK, HC], BF16)
            o_all = cst.tile([128, 2 * NBLK], F32)

            # zero tail rows of the last (partial) block so 0*NaN can't leak
            # into matmul accumulations
            ltail = NPC - (NBLK - 1) * BLOCK
            if ltail < 128:
                for tb in (xr0_sb, xr1_sb, xr2_sb, h1_sb, h2_sb):
                    nc.vector.memset(tb[:, NBLK - 1, :], 0.0)

            def cs(name):
                kind, r, c0i, w = COLS[name]
                return (CF if kind == "f" else CB)[0:r, c0i:c0i + w]

            ident = cs("ident")

            # ---------------- phase A: L0 node prep ----------------
            for b in range(NBLK):
                st = b * BLOCK
                sz = min(BLOCK, NPC - st)
                xTc = sb.tile([D_IN, 128], BF16, tag="xTc")
                nc.sync.dma_start(xTc[:, :sz], xT[:, st:st + sz])
                p1 = psa.tile([64, 128], F32, tag="prep", space="PSUM")
                nc.tensor.matmul(p1[:, :sz], lhsT=cs("W_in"), rhs=xTc[:, :sz],
                                 start=True, stop=True)
                h0T = sb.tile([64, 128], BF16, tag="h0T")
                nc.scalar.activation(h0T[:, :sz], p1[:, :sz], AF.Relu,
                                     bias=cs("b_in"))
                p2 = psa.tile([128, HC], F32, tag="prep", space="PSUM")
                nc.tensor.matmul(p2[:sz, :], lhsT=h0T[:, :sz], rhs=cs("Wl0"),
                                 start=True, stop=True)
                cp = sb.tile([128, HC], BF16, tag="cpA")
                nc.scalar.copy(cp[:sz, :], p2[:sz, :])
                nc.sync.dma_start(xl0_own[st:st + sz, :], cp[:sz, :])
                p3 = psa.tile([128, HC], F32, tag="prep", space="PSUM")
                nc.tensor.matmul(p3[:sz, :], lhsT=h0T[:, :sz], rhs=cs("Wr0"),
                                 start=True, stop=True)
                nc.scalar.copy(xr0_sb[:sz, b, :], p3[:sz, :])

            nc.gpsimd.collective_compute(
                "AllGather", ALU.bypass, ins=[xl0_own[:]], outs=[xl0_full[:]],
                replica_groups=rg)

            # ---------------- edge pass for layers 0/1 ----------------
            def edge_pass(lidx, xl_full, xr_sb, attB_name, aB_name, bB_name):
                """lidx 0: h1_sb[b] = relu(bn(gat)); interleave phase C chunk.
                lidx 1: h2_sb[b] = relu(bn(gat)) + h1_sb[b]; interleave E."""
                for b in range(NBLK):
                    st = b * BLOCK
                    nreal = min(BLOCK, NPC - st)
                    tl = blk_tiles[b]
                    T_all = sum(T for _, _, T in tl)
                    # gathers (xl only; xr is SBUF-resident)
                    gts = []
                    for (tg, nm, T) in tl:
                        g = gat.tile([128, T, HC], BF16, tag=f"gxl_{nm}")
                        src_ap = xl_full[0:HALF, :] if nm == "lo" else xl_full[HALF:N, :]
                        _gather(nc, g, src_ap, ixl_t, tg, T, HC)
                        gts.append((g, tg, T))
                    acc = ps.tile([128, HEADS, HID + 1], F32, tag="acc",
                                  space="PSUM")
                    tloc = 0
                    for g, tg0, T in gts:
                        for t in range(T):
                            gt = tg0 + t
                            xl_s = g[:, t, :]
                            oh = sb.tile([128, 128], BF16, tag="oh")
                            nc.vector.tensor_scalar(
                                out=oh[:], in0=cs("iotaB"),
                                scalar1=dstl_t[:, gt:gt + 1], scalar2=None,
                                op0=ALU.is_equal)
                            ohT = sb.tile([128, 128], BF16, tag="ohT")
                            nc.sync.dma_start_transpose(ohT[:], oh[:])
                            u = psu.tile([128, HC], F32, tag="u", space="PSUM")
                            nc.tensor.matmul(u[:], lhsT=ohT[:],
                                             rhs=xr_sb[:, b, :],
                                             start=True, stop=False)
                            nc.tensor.matmul(u[:], lhsT=ident, rhs=xl_s,
                                             start=False, stop=True)
                            v = sb.tile([128, HEADS, HID], BF16, tag="v")
                            _prelu(nc, sb,
                                   v[:].rearrange("a h c -> a (h c)"), u[:],
                                   [128, HC])
                            lg = sb.tile([128, HEADS], F32, tag="lg")
                            pscr = sb.tile([128, HEADS, HID], BF16, tag="pscr")
                            attB = cs(attB_name)
                            nc.vector.tensor_tensor(
                                out=pscr[:].rearrange("a h c -> a (h c)"),
                                in0=v[:].rearrange("a h c -> a (h c)"),
                                in1=attB, op=ALU.mult)
                            nc.vector.tensor_reduce(
                                out=lg[:], in_=pscr[:],
                                axis=mybir.AxisListType.X, op=ALU.add)
                            ex = sb.tile([128, HEADS], F32, tag="ex")
                            nc.scalar.activation(ex[:], lg[:], AF.Exp)
                            rhs = sb.tile([128, HEADS, HID + 1], BF16,
                                          tag="rhs")
                            nc.scalar.copy(rhs[:, :, HID:HID + 1],
                                           ex[:, :, None])
                            nc.vector.tensor_tensor(
                                out=rhs[:, :, 0:HID],
                                in0=g[:, t, :].rearrange(
                                    "a (h c) -> a h c", h=HEADS),
                                in1=ex[:, :, None].to_broadcast(
                                    [128, HEADS, HID]),
                                op=ALU.mult)
                            nc.tensor.matmul(
                                acc[:].rearrange("a h c -> a (h c)"),
                                lhsT=oh[:],
                                rhs=rhs[:].rearrange("a h c -> a (h c)"),
                                start=(tloc == 0), stop=(tloc == T_all - 1))
                            tloc += 1
                    # ---- block post ----
                    den = sb.tile([128, HEADS], F32, tag="den")
                    nc.vector.tensor_scalar(out=den[:], in0=acc[:, :, HID:HID + 1],
                                            scalar1=1e-20, scalar2=None,
                                            op0=ALU.add)
                    rc = sb.tile([128, HEADS], F32, tag="rc")
                    nc.vector.reciprocal(rc[:], den[:])
                    go = sb.tile([128, HEADS, HID], BF16, tag="go")
                    nc.vector.tensor_tensor(
                        out=go[:], in0=acc[:, :, 0:HID],
                        in1=rc[:, :, None].to_broadcast([128, HEADS, HID]),
                        op=ALU.mult)
                    t1 = sb.tile([128, HC], BF16, tag="t1")
                    nc.vector.tensor_tensor(
                        out=t1[:], in0=go[:].rearrange("a h c -> a (h c)"),
                        in1=cs(aB_name), op=ALU.mult)
                    t2 = sb.tile([128, HC], BF16, tag="t2")
                    nc.vector.tensor_tensor(out=t2[:], in0=t1[:],
                                            in1=cs(bB_name), op=ALU.add)
                    if lidx == 0:
                        nc.scalar.activation(h1_sb[:nreal, b, :],
                                             t2[:nreal, :], AF.Relu)
                        # ---- phase C chunk b: L1 node prep ----
                        hcur, Wk, xr_next = (
                            h1_sb, ("Wl1k0", "Wl1k1", "Wr1k0", "Wr1k1"),
                            xr1_sb)
                    else:
                        hr = sb.tile([128, HC], BF16, tag="hr")
                        nc.scalar.activation(hr[:nreal, :], t2[:nreal, :],
                                             AF.Relu)
                        nc.vector.tensor_tensor(out=h2_sb[:nreal, b, :],
                                                in0=hr[:nreal, :],
                                                in1=h1_sb[:nreal, b, :],
                                                op=ALU.add)
                        hcur, Wk, xr_next = (
                            h2_sb, ("Wl2k0", "Wl2k1", "Wr2k0", "Wr2k1"),
                            xr2_sb)
                    # interleaved next-layer node prep for this block
                    ht = []
                    for half in range(2):
                        tph = pst.tile([128, 128], BF16, tag="tp",
                                       space="PSUM")
                        nc.tensor.transpose(
                            tph[:], hcur[:, b, half * 128:(half + 1) * 128],
                            ident)
                        hth = sb.tile([128, 128], BF16, tag=f"ht{half}")
                        nc.scalar.copy(hth[:], tph[:])
                        ht.append(hth)
                    ocols = HC if lidx == 0 else OUT
                    pl = psa.tile([128, HC], F32, tag="prep", space="PSUM")
                    nc.tensor.matmul(pl[:nreal, 0:ocols],
                                     lhsT=ht[0][:, :nreal],
                                     rhs=cs(Wk[0]), start=True, stop=False)
                    nc.tensor.matmul(pl[:nreal, 0:ocols],
                                     lhsT=ht[1][:, :nreal],
                                     rhs=cs(Wk[1]), start=False, stop=True)
                    if lidx == 0:
                        cpl = sb.tile([128, HC], BF16, tag="cpA")
                        nc.scalar.copy(cpl[:nreal, :], pl[:nreal, 0:ocols])
                        nc.sync.dma_start(xl1_own[st:st + nreal, :],
                                          cpl[:nreal, :])
                    else:
                        cpl = sb.tile([128, 128], BF16, tag="cpE")
                        nc.vector.memset(cpl[:], 0.0)
                        nc.scalar.copy(cpl[:nreal, 0:OUT], pl[:nreal, 0:ocols])
                        nc.sync.dma_start(xl2_own[st:st + nreal, :],
                                          cpl[:nreal, :])
                    pr = psa.tile([128, HC], F32, tag="prep", space="PSUM")
                    nc.tensor.matmul(pr[:nreal, 0:ocols],
                                     lhsT=ht[0][:, :nreal],
                                     rhs=cs(Wk[2]), start=True, stop=False)
                    nc.tensor.matmul(pr[:nreal, 0:ocols],
                                     lhsT=ht[1][:, :nreal],
                                     rhs=cs(Wk[3]), start=False, stop=True)
                    nc.scalar.copy(xr_next[:nreal, b, 0:ocols],
                                   pr[:nreal, 0:ocols])

            edge_pass(0, xl0_full, xr0_sb, "attB0", "aB0", "bB0")
            nc.gpsimd.collective_compute(
                "AllGather", ALU.bypass, ins=[xl1_own[:]], outs=[xl1_full[:]],
                replica_groups=rg)
            edge_pass(1, xl1_full, xr1_sb, "attB1", "aB1", "bB1")
            nc.gpsimd.collective_compute(
                "AllGather", ALU.bypass, ins=[xl2_own[:]], outs=[xl2_full[:]],
                replica_groups=rg)

            # ---------------- L2 edge pass (1 head, OUT=2) ----------------
            att2 = cs("attB2")
            for b in range(NBLK):
                st = b * BLOCK
                nreal = min(BLOCK, NPC - st)
                tl = blk_tiles[b]
                T_all = sum(T for _, _, T in tl)
                gts = []
                for (tg, nm, T) in tl:
                    g = gat.tile([128, T, 128], BF16, tag=f"g2_{nm}")
                    src_ap = xl2_full[0:HALF, :] if nm == "lo" else xl2_full[HALF:N, :]
                    _gather(nc, g, src_ap, ixl_t, tg, T, 128)
                    gts.append((g, tg, T))
                acc2 = ps.tile([128, OUT + 1], F32, tag="acc", space="PSUM")
                tloc = 0
                for g, tg0, T in gts:
                    for t in range(T):
                        gt = tg0 + t
                        oh = sb.tile([128, 128], BF16, tag="oh")
                        nc.vector.tensor_scalar(
                            out=oh[:], in0=cs("iotaB"),
                            scalar1=dstl_t[:, gt:gt + 1], scalar2=None,
                            op0=ALU.is_equal)
                        ohT = sb.tile([128, 128], BF16, tag="ohT")
                        nc.sync.dma_start_transpose(ohT[:], oh[:])
                        u2 = psu.tile([128, OUT], F32, tag="u", space="PSUM")
                        nc.tensor.matmul(u2[:], lhsT=ohT[:],
                                         rhs=xr2_sb[:, b, :],
                                         start=True, stop=False)
                        nc.tensor.matmul(u2[:], lhsT=ident,
                                         rhs=g[:, t, 0:OUT],
                                         start=False, stop=True)
                        v2 = sb.tile([128, OUT], BF16, tag="v2")
                        _prelu(nc, sb, v2[:], u2[:], [128, OUT])
                        lg2 = sb.tile([128, 1], F32, tag="lg2")
                        p2scr = sb.tile([128, 1, OUT], BF16, tag="p2scr")
                        nc.vector.tensor_tensor(
                            out=p2scr[:, 0, :], in0=v2[:], in1=att2[:, 0:OUT],
                            op=ALU.mult)
                        nc.vector.tensor_reduce(
                            out=lg2[:], in_=p2scr[:],
                            axis=mybir.AxisListType.X, op=ALU.add)
                        ex2 = sb.tile([128, 1], F32, tag="ex2")
                        nc.scalar.activation(ex2[:], lg2[:], AF.Exp)
                        rhs2 = sb.tile([128, OUT + 1], BF16, tag="rhs2")
                        nc.scalar.copy(rhs2[:, OUT:OUT + 1], ex2[:])
                        nc.vector.tensor_scalar(
                            out=rhs2[:, 0:OUT], in0=g[:, t, 0:OUT],
                            scalar1=ex2[:], scalar2=None, op0=ALU.mult)
                        nc.tensor.matmul(acc2[:], lhsT=oh[:], rhs=rhs2[:],
                                         start=(tloc == 0),
                                         stop=(tloc == T_all - 1))
                        tloc += 1
                den2 = sb.tile([128, 1], F32, tag="den2")
                nc.vector.tensor_scalar(out=den2[:], in0=acc2[:, OUT:OUT + 1],
                                        scalar1=1e-20, scalar2=None, op0=ALU.add)
                rc2 = sb.tile([128, 1], F32, tag="rc2")
                nc.vector.reciprocal(rc2[:], den2[:])
                o2 = sb.tile([128, OUT], F32, tag="o2")
                nc.vector.tensor_scalar(out=o2[:], in0=acc2[:, 0:OUT],
                                        scalar1=rc2[:], scalar2=None,
                                        op0=ALU.mult)
                nc.vector.tensor_tensor(out=o_all[:, 2 * b:2 * b + 2],
                                        in0=o2[:], in1=cs("bias2F"),
                                        op=ALU.add)

            # ---------------- phase G: log_softmax ----------------
            for b in range(NBLK):
                st = b * BLOCK
                nreal = min(BLOCK, NPC - st)
                d = sb.tile([128, 1], F32, tag="d")
                nc.vector.tensor_tensor(out=d[:],
                                        in0=o_all[:, 2 * b + 1:2 * b + 2],
                                        in1=o_all[:, 2 * b:2 * b + 1],
                                        op=ALU.subtract)
                e = sb.tile([128, 1], F32, tag="e")
                nc.scalar.activation(e[:], d[:], AF.Exp)
                ep1 = sb.tile([128, 1], F32, tag="ep1")
                nc.vector.tensor_scalar(out=ep1[:], in0=e[:], scalar1=1.0,
                                        scalar2=None, op0=ALU.add)
                l = sb.tile([128, 1], F32, tag="l")
                nc.scalar.activation(l[:], ep1[:], AF.Ln)
                ls = sb.tile([128, 2], F32, tag="ls")
                nc.vector.tensor_scalar(out=ls[:, 0:1], in0=l[:], scalar1=-1.0,
                                        scalar2=None, op0=ALU.mult)
                nc.vector.tensor_tensor(out=ls[:, 1:2], in0=d[:], in1=l[:],
                                        op=ALU.subtract)
                nc.sync.dma_start(out[st:st + nreal, :], ls[:nreal, :])

    nc.compile()
    return nc


COLS = None  # set by kernel()


# ---------------------------------------------------------------- entry
_CACHE = {}
LAST_RESULTS = None


def kernel(**inputs):
    global COLS
    x = np.asarray(inputs["x"], np.float32)
    ei = np.asarray(inputs["edge_index"]).astype(np.int64)
    N = x.shape[0]
    NPC = N // NCORES

    idx_xl, dstl, schedule, NBLK = preprocess(ei, N)
    TT = sum(T for _, _, T in schedule)
    cf, cb, COLS = pack_consts(inputs)
    CWF, CWB = cf.shape[1], cb.shape[1]

    key = (N, TT, NBLK, tuple(schedule))
    if key not in _CACHE:
        _CACHE[key] = build(N, schedule, NBLK, TT, CWF, CWB)
    nc = _CACHE[key]

    in_maps = []
    for c in range(NCORES):
        sl = slice(c * NPC, (c + 1) * NPC)
        in_maps.append(dict(
            xT=np.ascontiguousarray(x[sl].T).astype(BFNP),
            idx_xl=idx_xl[c], dstl=dstl[c], cf=cf, cb=cb,
        ))
    res = run_bass_kernel_spmd(nc, in_maps, list(range(NCORES)))
    global LAST_RESULTS
    LAST_RESULTS = res
    outs = [res.results[c]["out"] for c in range(NCORES)]
    return np.concatenate(outs, axis=0).astype(np.float32)


# revision 16
# speedup vs baseline: 1.7917x; 1.7917x over previous
"""Trainium2 Bass kernel for EnhancedGATModel (3-layer GATv2, N=50000, E=800000).

v2 strategy (8 NeuronCores, graph-partitioned by destination node):
- Host: append self-loops, sort edges by dst, partition dst nodes 6250/core,
  bucket edges per 128-dst block, split each block's edges by src half
  (int16 gather indices), pad to 128-edge tiles uniformly across cores.
- Device (single SPMD NEFF on 8 cores), all-bf16 edge path:
  * per-layer node tables xl = h@Wl (own shard, bf16) + AllGather -> full
    table; xr tables and h residuals stay SBUF-resident (never leave chip)
  * per tile: dma_gather of xl[src] rows (bf16); one-hot (iota==dstl) on DVE;
    xr[dst] rows reconstructed on the PE via ohT@xr_block (no xr gather);
    u = xr_e + xl_s accumulated in PSUM; Prelu on Act; per-head logits via
    mult+reduce on DVE; exp on Act; weighted scatter-add via one-hot
    matmul accumulating [num|den] in PSUM across the block's tiles
  * block post: normalize, BN affine (folded), relu, residual; next layer's
    node prep (xl/xr matmuls) interleaved per block; final log_softmax.
"""
import sys
import numpy as np
import ml_dtypes

sys.path.insert(0, "/opt/trn_rl_repo")

import concourse.bass as bass
import concourse.mybir as mybir
import concourse.tile as tile
from concourse import bacc
from concourse.bass_utils import run_bass_kernel_spmd

F32 = mybir.dt.float32
BF16 = mybir.dt.bfloat16
I16 = mybir.dt.int16
AF = mybir.ActivationFunctionType
ALU = mybir.AluOpType
BFNP = ml_dtypes.bfloat16

NCORES = 8
BLOCK = 128
D_IN, HID, HEADS, OUT = 128, 64, 4, 2
HC = HEADS * HID  # 256
NEG_SLOPE = 0.2
BN_EPS = 1e-5
GMAX = 8  # dma_gather limit: <=1024 indices per op
PRELU_VIA_DVE = False  # CoreSim lacks Prelu; flip for simulation


def _prelu(nc, sb, out_ap, in_ap, shape):
    """out = prelu(in). Act Prelu on hw; mult+max on DVE for CoreSim."""
    if not PRELU_VIA_DVE:
        nc.scalar.activation(out_ap, in_ap, AF.Prelu, alpha=NEG_SLOPE)
    else:
        w = sb.tile(shape, F32, tag="pw")
        nc.vector.tensor_scalar(out=w[:], in0=in_ap, scalar1=NEG_SLOPE,
                                scalar2=None, op0=ALU.mult)
        nc.vector.tensor_tensor(out=out_ap, in0=w[:], in1=in_ap, op=ALU.max)


# ---------------------------------------------------------------- host prep
def preprocess(edge_index, N):
    """Per-core gather index / dst-local arrays and the tile schedule.

    schedule: list of (block, 'lo'|'hi', ntiles) in tile order; uniform
    across cores. Edge k of a (core,block,half) group lands at partition
    k%128 of tile k//128; pads use src-index 0 (finite reads) and
    dst_local=200 (masked out of the one-hot).
    """
    NPC = N // NCORES
    NBLK = (NPC + BLOCK - 1) // BLOCK
    HALF = N // 2
    src = np.concatenate([edge_index[0], np.arange(N)]).astype(np.int64)
    dst = np.concatenate([edge_index[1], np.arange(N)]).astype(np.int64)
    order = np.argsort(dst, kind="stable")
    src, dst = src[order], dst[order]
    core_of = dst // NPC
    groups = {}
    for c in range(NCORES):
        m = core_of == c
        sc, dc = src[m], dst[m]
        loc = dc - c * NPC
        blk = loc // BLOCK
        lo = sc < HALF
        for b in range(NBLK):
            mb = blk == b
            groups[(c, b, 0)] = (sc[mb & lo], loc[mb & lo] % BLOCK)
            groups[(c, b, 1)] = (sc[mb & ~lo] - HALF, loc[mb & ~lo] % BLOCK)
    schedule = []
    for b in range(NBLK):
        for h, nm in ((0, "lo"), (1, "hi")):
            mx = max(len(groups[(c, b, h)][0]) for c in range(NCORES))
            T = (mx + 127) // 128
            if T > 0:
                schedule.append((b, nm, T))
    TT = sum(T for _, _, T in schedule)
    idx_xl = np.zeros((NCORES, 128, 8 * TT), np.int16)
    dstl = np.full((NCORES, 128, TT), 200.0, np.float32)
    t0 = 0
    for b, nm, T in schedule:
        h = 0 if nm == "lo" else 1
        for c in range(NCORES):
            s, dl = groups[(c, b, h)]
            ne = len(s)
            pad = T * 128 - ne
            sp = np.concatenate([s, np.zeros(pad, np.int64)]).astype(np.int64)
            dlp = np.concatenate([dl, np.full(pad, 200)]).astype(np.int64)
            wrap = sp.reshape(8 * T, 16).T.astype(np.int16)
            idx_xl[c, :, 8 * t0:8 * (t0 + T)] = np.tile(wrap, (8, 1))
            dstl[c, :, t0:t0 + T] = dlp.reshape(T, 128).T.astype(np.float32)
        t0 += T
    return idx_xl, dstl, schedule, NBLK


def pack_consts(ip):
    """Pack constants into cf (f32) and cb (bf16) [128, *] tensors.
    Returns (cf_arr, cb_arr, dict name -> ('f'|'b', rows, col0, cols))."""
    cols = {}
    fparts, bparts = [], []
    fc, bc = [0], [0]

    def addf(name, arr):
        a = np.zeros((128, arr.shape[1]), np.float32)
        a[:arr.shape[0]] = arr
        cols[name] = ("f", arr.shape[0], fc[0], arr.shape[1])
        fparts.append(a)
        fc[0] += arr.shape[1]

    def addb(name, arr):
        a = np.zeros((128, arr.shape[1]), np.float32)
        a[:arr.shape[0]] = arr
        cols[name] = ("b", arr.shape[0], bc[0], arr.shape[1])
        bparts.append(a)
        bc[0] += arr.shape[1]

    bcast = lambda v: np.broadcast_to(
        np.asarray(v, np.float32)[None, :], (128, len(np.asarray(v)))).copy()

    # --- bf16 consts
    addb("ident", np.eye(128, dtype=np.float32))
    iota = np.broadcast_to(np.arange(128, dtype=np.float32), (128, 128))
    addb("iotaB", np.ascontiguousarray(iota))
    addb("attB0", bcast(np.asarray(ip["att0"], np.float32).reshape(-1)))
    addb("attB1", bcast(np.asarray(ip["att1"], np.float32).reshape(-1)))
    addb("attB2", bcast(np.asarray(ip["att2"], np.float32).reshape(-1)))
    g, bt = np.asarray(ip["bn_gamma"]), np.asarray(ip["bn_beta"])
    mu, var = np.asarray(ip["bn_mean"]), np.asarray(ip["bn_var"])
    for l in range(2):
        a = g[l] / np.sqrt(var[l] + BN_EPS)
        bias_l = np.asarray(ip[f"bias{l}"], np.float32)
        b = bt[l] - mu[l] * a + a * bias_l
        addb(f"aB{l}", bcast(a))
        addb(f"bB{l}", bcast(b))
    addb("W_in", np.asarray(ip["W_in"], np.float32))          # [128, 64]
    addb("Wl0", np.asarray(ip["Wl0"], np.float32))            # [64, 256]
    addb("Wr0", np.asarray(ip["Wr0"], np.float32))
    Wl1 = np.asarray(ip["Wl1"], np.float32)
    Wr1 = np.asarray(ip["Wr1"], np.float32)
    addb("Wl1k0", Wl1[:128]); addb("Wl1k1", Wl1[128:])
    addb("Wr1k0", Wr1[:128]); addb("Wr1k1", Wr1[128:])
    Wl2 = np.asarray(ip["Wl2"], np.float32)
    Wr2 = np.asarray(ip["Wr2"], np.float32)
    addb("Wl2k0", Wl2[:128]); addb("Wl2k1", Wl2[128:])
    addb("Wr2k0", Wr2[:128]); addb("Wr2k1", Wr2[128:])
    # --- f32 consts
    addf("b_in", np.asarray(ip["b_in"], np.float32).reshape(-1, 1))  # [64,1]
    addf("bias2F", bcast(np.asarray(ip["bias2"], np.float32)))       # [128,2]

    cf = np.concatenate(fparts, axis=1) if fparts else np.zeros((128, 1), np.float32)
    cb = np.concatenate(bparts, axis=1)
    return cf, cb.astype(BFNP), cols


def _gather(nc, out_tile, in_ap, idx_tile, tstart, T, elem):
    """Chunked dma_gather: out_tile[:, k:k+Tc, :] = table[idx[tile k..]]."""
    k = 0
    while k < T:
        Tc = min(GMAX, T - k)
        nc.gpsimd.dma_gather(
            out_ap=out_tile[:, k:k + Tc, :], in_ap=in_ap,
            idxs_ap=idx_tile[:, 8 * (tstart + k):8 * (tstart + k + Tc)],
            num_idxs=128 * Tc, num_idxs_reg=128 * Tc, elem_size=elem)
        k += Tc


def build(N, schedule, NBLK, TT, CWF, CWB):
    NPC = N // NCORES
    HALF = N // 2
    nc = bacc.Bacc("TRN2", target_bir_lowering=False, debug=False)

    xT = nc.dram_tensor("xT", [D_IN, NPC], BF16, kind="ExternalInput")
    idx_xl = nc.dram_tensor("idx_xl", [128, 8 * TT], I16, kind="ExternalInput")
    dstl = nc.dram_tensor("dstl", [128, TT], F32, kind="ExternalInput")
    cfd = nc.dram_tensor("cf", [128, CWF], F32, kind="ExternalInput")
    cbd = nc.dram_tensor("cb", [128, CWB], BF16, kind="ExternalInput")
    out = nc.dram_tensor("out", [NPC, OUT], F32, kind="ExternalOutput")

    xl0_own = nc.dram_tensor("xl0_own", [NPC, HC], BF16)
    xl0_full = nc.dram_tensor("xl0_full", [N, HC], BF16, addr_space="Shared")
    xl1_own = nc.dram_tensor("xl1_own", [NPC, HC], BF16)
    xl1_full = nc.dram_tensor("xl1_full", [N, HC], BF16, addr_space="Shared")
    xl2_own = nc.dram_tensor("xl2_own", [NPC, 128], BF16)
    xl2_full = nc.dram_tensor("xl2_full", [N, 128], BF16, addr_space="Shared")

    # block tile layout: per block b, list of (tile_start, 'lo'|'hi', T)
    blk_tiles = {b: [] for b in range(NBLK)}
    t0 = 0
    for b, nm, T in schedule:
        blk_tiles[b].append((t0, nm, T))
        t0 += T

    rg = [list(range(NCORES))]

    with tile.TileContext(nc) as tc:
        import contextlib
        with contextlib.ExitStack() as ctx:
            cst = ctx.enter_context(tc.tile_pool(name="cst", bufs=1))
            sb = ctx.enter_context(tc.tile_pool(name="sb", bufs=3))
            gat = ctx.enter_context(tc.tile_pool(name="gat", bufs=2))
            ps = ctx.enter_context(tc.tile_pool(name="ps", bufs=2, space="PSUM"))
            psu = ctx.enter_context(tc.tile_pool(name="psu", bufs=2, space="PSUM"))
            pst = ctx.enter_context(tc.tile_pool(name="pst", bufs=2, space="PSUM"))
            psa = ctx.enter_context(tc.tile_pool(name="psa", bufs=2, space="PSUM"))

            CF = cst.tile([128, CWF], F32)
            nc.sync.dma_start(CF[:], cfd[:])
            CB = cst.tile([128, CWB], BF16)
            nc.sync.dma_start(CB[:], cbd[:])
            ixl_t = cst.tile([128, 8 * TT], I16)
            nc.sync.dma_start(ixl_t[:], idx_xl[:])
            dstl_t = cst.tile([128, TT], F32)
            nc.sync.dma_start(dstl_t[:], dstl[:])

            xr0_sb = cst.tile([128, NBLK, HC], BF16)
            xr1_sb = cst.tile([128, NBLK, HC], BF16)
            xr2_sb = cst.tile([128, NBLK, OUT], BF16)
            h1_sb = cst.tile([128, NBLK, HC], BF16)
            h2_sb = cst.tile([128, NBLK, HC], BF16)
            o_all = cst.tile([128, 2 * NBLK], F32)

            # zero tail rows of the last (partial) block so 0*NaN can't leak
            # into matmul accumulations
            ltail = NPC - (NBLK - 1) * BLOCK
            if ltail < 128:
                for tb in (xr0_sb, xr1_sb, xr2_sb, h1_sb, h2_sb):
                    nc.vector.memset(tb[:, NBLK - 1, :], 0.0)

            def cs(name):
                kind, r, c0i, w = COLS[name]
                return (CF if kind == "f" else CB)[0:r, c0i:c0i + w]

            ident = cs("ident")

            # ---------------- phase A: L0 node prep ----------------
            for b in range(NBLK):
                st = b * BLOCK
                sz = min(BLOCK, NPC - st)
                xTc = sb.tile([D_IN, 128], BF16, tag="xTc")
                nc.sync.dma_start(xTc[:, :sz], xT[:, st:st + sz])
                p1 = psa.tile([64, 128], F32, tag="prep", space="PSUM")
                nc.tensor.matmul(p1[:, :sz], lhsT=cs("W_in"), rhs=xTc[:, :sz],
                                 start=True, stop=True)
                h0T = sb.tile([64, 128], BF16, tag="h0T")
                nc.scalar.activation(h0T[:, :sz], p1[:, :sz], AF.Relu,
                                     bias=cs("b_in"))
                p2 = psa.tile([128, HC], F32, tag="prep", space="PSUM")
                nc.tensor.matmul(p2[:sz, :], lhsT=h0T[:, :sz], rhs=cs("Wl0"),
                                 start=True, stop=True)
                cp = sb.tile([128, HC], BF16, tag="cpA")
                nc.scalar.copy(cp[:sz, :], p2[:sz, :])
                nc.sync.dma_start(xl0_own[st:st + sz, :], cp[:sz, :])
                p3 = psa.tile([128, HC], F32, tag="prep", space="PSUM")
                nc.tensor.matmul(p3[:sz, :], lhsT=h0T[:, :sz], rhs=cs("Wr0"),
                                 start=True, stop=True)
                nc.scalar.copy(xr0_sb[:sz, b, :], p3[:sz, :])

            nc.gpsimd.collective_compute(
                "AllGather", ALU.bypass, ins=[xl0_own[:]], outs=[xl0_full[:]],
                replica_groups=rg)

            # ---------------- edge pass for layers 0/1 ----------------
            def edge_pass(lidx, xl_full, xr_sb, attB_name, aB_name, bB_name):
                """lidx 0: h1_sb[b] = relu(bn(gat)); interleave phase C chunk.
                lidx 1: h2_sb[b] = relu(bn(gat)) + h1_sb[b]; interleave E."""
                for b in range(NBLK):
                    st = b * BLOCK
                    nreal = min(BLOCK, NPC - st)
                    tl = blk_tiles[b]
                    T_all = sum(T for _, _, T in tl)
                    # gathers (xl only; xr is SBUF-resident)
                    gts = []
                    for (tg, nm, T) in tl:
                        g = gat.tile([128, T, HC], BF16, tag=f"gxl_{nm}")
                        src_ap = xl_full[0:HALF, :] if nm == "lo" else xl_full[HALF:N, :]
                        _gather(nc, g, src_ap, ixl_t, tg, T, HC)
                        gts.append((g, tg, T))
                    acc = ps.tile([128, HEADS, HID + 1], F32, tag="acc",
                                  space="PSUM")
                    tloc = 0
                    for g, tg0, T in gts:
                        for t in range(T):
                            gt = tg0 + t
                            xl_s = g[:, t, :]
                            oh = sb.tile([128, 128], BF16, tag="oh")
                            nc.vector.tensor_scalar(
                                out=oh[:], in0=cs("iotaB"),
                                scalar1=dstl_t[:, gt:gt + 1], scalar2=None,
                                op0=ALU.is_equal)
                            tp = pst.tile([128, 128], BF16, tag="tp",
                                          space="PSUM")
                            nc.tensor.transpose(tp[:], oh[:], ident)
                            ohT = sb.tile([128, 128], BF16, tag="ohT")
                            nc.scalar.copy(ohT[:], tp[:])
                            u = psu.tile([128, HC], F32, tag="u", space="PSUM")
                            nc.tensor.matmul(u[:], lhsT=ohT[:],
                                             rhs=xr_sb[:, b, :],
                                             start=True, stop=False)
                            nc.tensor.matmul(u[:], lhsT=ident, rhs=xl_s,
                                             start=False, stop=True)
                            v = sb.tile([128, HEADS, HID], BF16, tag="v")
                            _prelu(nc, sb,
                                   v[:].rearrange("a h c -> a (h c)"), u[:],
                                   [128, HC])
                            lg = sb.tile([128, HEADS], F32, tag="lg")
                            pscr = sb.tile([128, HEADS, HID], BF16, tag="pscr")
                            attB = cs(attB_name)
                            nc.vector.tensor_tensor(
                                out=pscr[:].rearrange("a h c -> a (h c)"),
                                in0=v[:].rearrange("a h c -> a (h c)"),
                                in1=attB, op=ALU.mult)
                            nc.vector.tensor_reduce(
                                out=lg[:], in_=pscr[:],
                                axis=mybir.AxisListType.X, op=ALU.add)
                            ex = sb.tile([128, HEADS], F32, tag="ex")
                            nc.scalar.activation(ex[:], lg[:], AF.Exp)
                            rhs = sb.tile([128, HEADS, HID + 1], BF16,
                                          tag="rhs")
                            nc.scalar.copy(rhs[:, :, HID:HID + 1],
                                           ex[:, :, None])
                            nc.vector.tensor_tensor(
                                out=rhs[:, :, 0:HID],
                                in0=g[:, t, :].rearrange(
                                    "a (h c) -> a h c", h=HEADS),
                                in1=ex[:, :, None].to_broadcast(
                                    [128, HEADS, HID]),
                                op=ALU.mult)
                            nc.tensor.matmul(
                                acc[:].rearrange("a h c -> a (h c)"),
                                lhsT=oh[:],
                                rhs=rhs[:].rearrange("a h c -> a (h c)"),
                                start=(tloc == 0), stop=(tloc == T_all - 1))
                            tloc += 1
                    # ---- block post ----
                    den = sb.tile([128, HEADS], F32, tag="den")
                    nc.vector.tensor_scalar(out=den[:], in0=acc[:, :, HID:HID + 1],
                                            scalar1=1e-20, scalar2=None,
                                            op0=ALU.add)
                    rc = sb.tile([128, HEADS], F32, tag="rc")
                    nc.vector.reciprocal(rc[:], den[:])
                    go = sb.tile([128, HEADS, HID], BF16, tag="go")
                    nc.vector.tensor_tensor(
                        out=go[:], in0=acc[:, :, 0:HID],
                        in1=rc[:, :, None].to_broadcast([128, HEADS, HID]),
                        op=ALU.mult)
                    t1 = sb.tile([128, HC], BF16, tag="t1")
                    nc.vector.tensor_tensor(
                        out=t1[:], in0=go[:].rearrange("a h c -> a (h c)"),
                        in1=cs(aB_name), op=ALU.mult)
                    t2 = sb.tile([128, HC], BF16, tag="t2")
                    nc.vector.tensor_tensor(out=t2[:], in0=t1[:],
                                            in1=cs(bB_name), op=ALU.add)
                    if lidx == 0:
                        nc.scalar.activation(h1_sb[:nreal, b, :],
                                             t2[:nreal, :], AF.Relu)
                        # ---- phase C chunk b: L1 node prep ----
                        hcur, Wk, xr_next = (
                            h1_sb, ("Wl1k0", "Wl1k1", "Wr1k0", "Wr1k1"),
                            xr1_sb)
                    else:
                        hr = sb.tile([128, HC], BF16, tag="hr")
                        nc.scalar.activation(hr[:nreal, :], t2[:nreal, :],
                                             AF.Relu)
                        nc.vector.tensor_tensor(out=h2_sb[:nreal, b, :],
                                                in0=hr[:nreal, :],
                                                in1=h1_sb[:nreal, b, :],
                                                op=ALU.add)
                        hcur, Wk, xr_next = (
                            h2_sb, ("Wl2k0", "Wl2k1", "Wr2k0", "Wr2k1"),
                            xr2_sb)
                    # interleaved next-layer node prep for this block
                    ht = []
                    for half in range(2):
                        tph = pst.tile([128, 128], BF16, tag="tp",
                                       space="PSUM")
                        nc.tensor.transpose(
                            tph[:], hcur[:, b, half * 128:(half + 1) * 128],
                            ident)
                        hth = sb.tile([128, 128], BF16, tag=f"ht{half}")
                        nc.scalar.copy(hth[:], tph[:])
                        ht.append(hth)
                    ocols = HC if lidx == 0 else OUT
                    pl = psa.tile([128, HC], F32, tag="prep", space="PSUM")
                    nc.tensor.matmul(pl[:nreal, 0:ocols],
                                     lhsT=ht[0][:, :nreal],
                                     rhs=cs(Wk[0]), start=True, stop=False)
                    nc.tensor.matmul(pl[:nreal, 0:ocols],
                                     lhsT=ht[1][:, :nreal],
                                     rhs=cs(Wk[1]), start=False, stop=True)
                    if lidx == 0:
                        cpl = sb.tile([128, HC], BF16, tag="cpA")
                        nc.scalar.copy(cpl[:nreal, :], pl[:nreal, 0:ocols])
                        nc.sync.dma_start(xl1_own[st:st + nreal, :],
                                          cpl[:nreal, :])
                    else:
                        cpl = sb.tile([128, 128], BF16, tag="cpE")
                        nc.vector.memset(cpl[:], 0.0)
                        nc.scalar.copy(cpl[:nreal, 0:OUT], pl[:nreal, 0:ocols])
                        nc.sync.dma_start(xl2_own[st:st + nreal, :],
                                          cpl[:nreal, :])
                    pr = psa.tile([128, HC], F32, tag="prep", space="PSUM")
                    nc.tensor.matmul(pr[:nreal, 0:ocols],
                                     lhsT=ht[0][:, :nreal],
                                     rhs=cs(Wk[2]), start=True, stop=False)
                    nc.tensor.matmul(pr[:nreal, 0:ocols],
                                     lhsT=ht[1][:, :nreal],
                                     rhs=cs(Wk[3]), start=False, stop=True)
                    nc.scalar.copy(xr_next[:nreal, b, 0:ocols],
                                   pr[:nreal, 0:ocols])

            edge_pass(0, xl0_full, xr0_sb, "attB0", "aB0", "bB0")
            nc.gpsimd.collective_compute(
                "AllGather", ALU.bypass, ins=[xl1_own[:]], outs=[xl1_full[:]],
                replica_groups=rg)
            edge_pass(1, xl1_full, xr1_sb, "attB1", "aB1", "bB1")
            nc.gpsimd.collective_compute(
                "AllGather", ALU.bypass, ins=[xl2_own[:]], outs=[xl2_full[:]],
                replica_groups=rg)

            # ---------------- L2 edge pass (1 head, OUT=2) ----------------
            att2 = cs("attB2")
            for b in range(NBLK):
                st = b * BLOCK
                nreal = min(BLOCK, NPC - st)
                tl = blk_tiles[b]
                T_all = sum(T for _, _, T in tl)
                gts = []
                for (tg, nm, T) in tl:
                    g = gat.tile([128, T, 128], BF16, tag=f"g2_{nm}")
                    src_ap = xl2_full[0:HALF, :] if nm == "lo" else xl2_full[HALF:N, :]
                    _gather(nc, g, src_ap, ixl_t, tg, T, 128)
                    gts.append((g, tg, T))
                acc2 = ps.tile([128, OUT + 1], F32, tag="acc", space="PSUM")
                tloc = 0
                for g, tg0, T in gts:
                    for t in range(T):
                        gt = tg0 + t
                        oh = sb.tile([128, 128], BF16, tag="oh")
                        nc.vector.tensor_scalar(
                            out=oh[:], in0=cs("iotaB"),
                            scalar1=dstl_t[:, gt:gt + 1], scalar2=None,
                            op0=ALU.is_equal)
                        tp = pst.tile([128, 128], BF16, tag="tp", space="PSUM")
                        nc.tensor.transpose(tp[:], oh[:], ident)
                        ohT = sb.tile([128, 128], BF16, tag="ohT")
                        nc.scalar.copy(ohT[:], tp[:])
                        u2 = psu.tile([128, OUT], F32, tag="u", space="PSUM")
                        nc.tensor.matmul(u2[:], lhsT=ohT[:],
                                         rhs=xr2_sb[:, b, :],
                                         start=True, stop=False)
                        nc.tensor.matmul(u2[:], lhsT=ident,
                                         rhs=g[:, t, 0:OUT],
                                         start=False, stop=True)
                        v2 = sb.tile([128, OUT], BF16, tag="v2")
                        _prelu(nc, sb, v2[:], u2[:], [128, OUT])
                        lg2 = sb.tile([128, 1], F32, tag="lg2")
                        p2scr = sb.tile([128, 1, OUT], BF16, tag="p2scr")
                        nc.vector.tensor_tensor(
                            out=p2scr[:, 0, :], in0=v2[:], in1=att2[:, 0:OUT],
                            op=ALU.mult)
                        nc.vector.tensor_reduce(
                            out=lg2[:], in_=p2scr[:],
                            axis=mybir.AxisListType.X, op=ALU.add)
                        ex2 = sb.tile([128, 1], F32, tag="ex2")
                        nc.scalar.activation(ex2[:], lg2[:], AF.Exp)
                        rhs2 = sb.tile([128, OUT + 1], BF16, tag="rhs2")
                        nc.scalar.copy(rhs2[:, OUT:OUT + 1], ex2[:])
                        nc.vector.tensor_scalar(
                            out=rhs2[:, 0:OUT], in0=g[:, t, 0:OUT],
                            scalar1=ex2[:], scalar2=None, op0=ALU.mult)
                        nc.tensor.matmul(acc2[:], lhsT=oh[:], rhs=rhs2[:],
                                         start=(tloc == 0),
                                         stop=(tloc == T_all - 1))
                        tloc += 1
                den2 = sb.tile([128, 1], F32, tag="den2")
                nc.vector.tensor_scalar(out=den2[:], in0=acc2[:, OUT:OUT + 1],
                                        scalar1=1e-20, scalar2=None, op0=ALU.add)
                rc2 = sb.tile([128, 1], F32, tag="rc2")
                nc.vector.reciprocal(rc2[:], den2[:])
                o2 = sb.tile([128, OUT], F32, tag="o2")
                nc.vector.tensor_scalar(out=o2[:], in0=acc2[:, 0:OUT],
                                        scalar1=rc2[:], scalar2=None,
                                        op0=ALU.mult)
                nc.vector.tensor_tensor(out=o_all[:, 2 * b:2 * b + 2],
                                        in0=o2[:], in1=cs("bias2F"),
                                        op=ALU.add)

            # ---------------- phase G: log_softmax ----------------
            for b in range(NBLK):
                st = b * BLOCK
                nreal = min(BLOCK, NPC - st)
                d = sb.tile([128, 1], F32, tag="d")
                nc.vector.tensor_tensor(out=d[:],
                                        in0=o_all[:, 2 * b + 1:2 * b + 2],
                                        in1=o_all[:, 2 * b:2 * b + 1],
                                        op=ALU.subtract)
                e = sb.tile([128, 1], F32, tag="e")
                nc.scalar.activation(e[:], d[:], AF.Exp)
                ep1 = sb.tile([128, 1], F32, tag="ep1")
                nc.vector.tensor_scalar(out=ep1[:], in0=e[:], scalar1=1.0,
                                        scalar2=None, op0=ALU.add)
                l = sb.tile([128, 1], F32, tag="l")
                nc.scalar.activation(l[:], ep1[:], AF.Ln)
                ls = sb.tile([128, 2], F32, tag="ls")
                nc.vector.tensor_scalar(out=ls[:, 0:1], in0=l[:], scalar1=-1.0,
                                        scalar2=None, op0=ALU.mult)
                nc.vector.tensor_tensor(out=ls[:, 1:2], in0=d[:], in1=l[:],
                                        op=ALU.subtract)
                nc.sync.dma_start(out[st:st + nreal, :], ls[:nreal, :])

    nc.compile()
    return nc


COLS = None  # set by kernel()


# ---------------------------------------------------------------- entry
_CACHE = {}
LAST_RESULTS = None


def kernel(**inputs):
    global COLS
    x = np.asarray(inputs["x"], np.float32)
    ei = np.asarray(inputs["edge_index"]).astype(np.int64)
    N = x.shape[0]
    NPC = N // NCORES

    idx_xl, dstl, schedule, NBLK = preprocess(ei, N)
    TT = sum(T for _, _, T in schedule)
    cf, cb, COLS = pack_consts(inputs)
    CWF, CWB = cf.shape[1], cb.shape[1]

    key = (N, TT, NBLK, tuple(schedule))
    if key not in _CACHE:
        _CACHE[key] = build(N, schedule, NBLK, TT, CWF, CWB)
    nc = _CACHE[key]

    in_maps = []
    for c in range(NCORES):
        sl = slice(c * NPC, (c + 1) * NPC)
        in_maps.append(dict(
            xT=np.ascontiguousarray(x[sl].T).astype(BFNP),
            idx_xl=idx_xl[c], dstl=dstl[c], cf=cf, cb=cb,
        ))
    res = run_bass_kernel_spmd(nc, in_maps, list(range(NCORES)))
    global LAST_RESULTS
    LAST_RESULTS = res
    outs = [res.results[c]["out"] for c in range(NCORES)]
    return np.concatenate(outs, axis=0).astype(np.float32)
